# revision 6
# baseline (speedup 1.0000x reference)
"""Trainium2 Bass kernel for the RGB-D cross-attention gate module.

Math shortcut: the module returns only gate = sigmoid(bn3(mlp2(relu(bn2(mlp1(gap))))))
where gap = spatial mean of (att_r + att_b + rgb + dep1).  Summing att_r over
spatial j gives proj_rgb @ s with s[i] = sum_j attn[i, j], so the full N x N
attention never needs materializing - only softmax column denominators d[j]
and the attention row-sum vector s.

Sharding: 8 cores = 4 samples x 2 column-halves of the attention (softmax is
over rows i, so a column shard is fully local; host adds the two 64-vector
partials per sample and applies the tiny MLP during gather).
"""

import numpy as np
import ml_dtypes

import concourse.bass as bass
import concourse.bacc as bacc
import concourse.mybir as mybir
import concourse.tile as tile
from concourse.bass_utils import run_bass_kernel_spmd

EPS = 1e-5
N = 4096          # spatial positions (64 x 64)
NH = 2048         # this core's attention-column half
C = 64            # channels
C1 = 256          # dep input channels
BF16 = mybir.dt.bfloat16
F32 = mybir.dt.float32
NPBF16 = ml_dtypes.bfloat16


def build_core_program():
    # Bacc (not plain Bass): its compile() pass legalizes multi-sem waits into
    # sequencer instructions; walrus rejects >1 sync wait per compute instr.
    nc = bacc.Bacc("TRN2", target_bir_lowering=False)

    # ---- DRAM I/O (per-core shards; all cores run this same program) ----
    rgb_d = nc.dram_tensor("rgb", (C, N), BF16, kind="ExternalInput")
    depA_d = nc.dram_tensor("depA", (128, N), BF16, kind="ExternalInput")
    depB_d = nc.dram_tensor("depB", (128, N), BF16, kind="ExternalInput")
    cwT0_d = nc.dram_tensor("cwT0", (128, C), BF16, kind="ExternalInput")
    cwT1_d = nc.dram_tensor("cwT1", (128, C), BF16, kind="ExternalInput")
    rgbwT_d = nc.dram_tensor("rgbwT", (C, C), BF16, kind="ExternalInput")
    depwT_d = nc.dram_tensor("depwT", (C, C), BF16, kind="ExternalInput")
    bn1_d = nc.dram_tensor("bn1", (C, 2), F32, kind="ExternalInput")
    out_d = nc.dram_tensor("out_vec", (C, 1), F32, kind="ExternalOutput")

    with tile.TileContext(nc) as tc:
        with (
            tc.tile_pool(name="consts", bufs=1) as consts,
            tc.tile_pool(name="big", bufs=1) as big,
            tc.tile_pool(name="pwork", bufs=3) as pwork,
            tc.tile_pool(name="small", bufs=4) as small,
            tc.tile_pool(name="ps", bufs=3, space="PSUM") as ps,
            tc.tile_pool(name="acc", bufs=1, space="PSUM") as acc,
        ):
            # ---- load inputs ----
            rgb_sb = big.tile([C, N], BF16, tag="rgb")
            depA = big.tile([128, N], BF16, tag="depA")
            depB = big.tile([128, N], BF16, tag="depB")
            nc.sync.dma_start(out=rgb_sb, in_=rgb_d.ap())
            nc.sync.dma_start(out=depA, in_=depA_d.ap())
            nc.sync.dma_start(out=depB, in_=depB_d.ap())

            cwT0 = consts.tile([128, C], BF16, tag="cwT0")
            cwT1 = consts.tile([128, C], BF16, tag="cwT1")
            rgbwT = consts.tile([C, C], BF16, tag="rgbwT")
            depwT = consts.tile([C, C], BF16, tag="depwT")
            bn1 = consts.tile([C, 2], F32, tag="bn1")
            nc.sync.dma_start(out=cwT0, in_=cwT0_d.ap())
            nc.sync.dma_start(out=cwT1, in_=cwT1_d.ap())
            nc.sync.dma_start(out=rgbwT, in_=rgbwT_d.ap())
            nc.sync.dma_start(out=depwT, in_=depwT_d.ap())
            nc.sync.dma_start(out=bn1, in_=bn1_d.ap())

            # ---- conv1x1(dep) + BN + ReLU -> dep1 (64, 4096) bf16 ----
            dep1 = big.tile([C, N], BF16, tag="dep1")
            rsumd = small.tile([C, 2], F32, tag="rsumd")
            for t in range(4):
                pc = ps.tile([128, 1024], F32, tag="ps")
                for u in range(2):
                    sl = slice(t * 1024 + u * 512, t * 1024 + (u + 1) * 512)
                    nc.tensor.matmul(pc[:C, u * 512:(u + 1) * 512], cwT0, depA[:, sl],
                                     start=True, stop=False)
                    nc.tensor.matmul(pc[:C, u * 512:(u + 1) * 512], cwT1, depB[:, sl],
                                     start=False, stop=True)
                nc.scalar.activation(
                    dep1[:, t * 1024:(t + 1) * 1024], pc[:C, :],
                    mybir.ActivationFunctionType.Relu,
                    bias=bn1[:, 1:2], scale=bn1[:, 0:1],
                    accum_out=rsumd[:, t:t + 1] if t < 2 else None,
                )

            # ---- projections: proj_rgb, proj_dep (64, 4096) bf16 ----
            prgb = big.tile([C, N], BF16, tag="prgb")
            pdep = big.tile([C, N], BF16, tag="pdep")
            for t in range(4):
                pr = ps.tile([128, 1024], F32, tag="ps")
                for u in range(2):
                    sl = slice(t * 1024 + u * 512, t * 1024 + (u + 1) * 512)
                    nc.tensor.matmul(pr[:C, u * 512:(u + 1) * 512], rgbwT, rgb_sb[:, sl],
                                     start=True, stop=True)
                nc.vector.tensor_copy(prgb[:, t * 1024:(t + 1) * 1024], pr[:C, :])
            for t in range(4):
                pd = ps.tile([128, 1024], F32, tag="ps")
                for u in range(2):
                    sl = slice(t * 1024 + u * 512, t * 1024 + (u + 1) * 512)
                    nc.tensor.matmul(pd[:C, u * 512:(u + 1) * 512], depwT, dep1[:, sl],
                                     start=True, stop=True)
                nc.vector.tensor_copy(pdep[:, t * 1024:(t + 1) * 1024], pd[:C, :])

            # ---- prsT[i, c] = (proj_rgb + proj_dep)^T, built by transposed matmuls ----
            prsT = big.tile([128, 32 * C], BF16, tag="prsT")
            for q in range(32):
                pt = ps.tile([128, 1024], F32, tag="ps")
                isl = slice(q * 128, (q + 1) * 128)
                nc.tensor.matmul(pt[:, :C], rgb_sb[:, isl], rgbwT, start=True, stop=False)
                nc.tensor.matmul(pt[:, :C], dep1[:, isl], depwT, start=False, stop=True)
                nc.vector.tensor_copy(prsT[:, q * C:(q + 1) * C], pt[:, :C])

            # ---- rsum_rgb = sum_{j<NH} rgb ----
            rsumr = small.tile([C, 1], F32, tag="rsumr")
            nc.vector.tensor_reduce(rsumr, rgb_sb[:, 0:NH], axis=mybir.AxisListType.X,
                                    op=mybir.AluOpType.add)

            # ---- main attention loop over 16 j-tiles of 128 columns ----
            s_psum = acc.tile([128, 32], F32, tag="sacc")
            for jt in range(16):
                P_t = pwork.tile([128, N], BF16, tag="P")
                dparts = small.tile([128, 4], F32, tag="dparts")
                jsl = slice(jt * 128, (jt + 1) * 128)
                for h in range(4):
                    pe = ps.tile([128, 1024], F32, tag="ps")
                    for u in range(2):
                        isl = slice(h * 1024 + u * 512, h * 1024 + (u + 1) * 512)
                        nc.tensor.matmul(pe[:, u * 512:(u + 1) * 512],
                                         pdep[:, jsl], prgb[:, isl],
                                         start=True, stop=True)
                    nc.scalar.activation(
                        P_t[:, h * 1024:(h + 1) * 1024], pe,
                        mybir.ActivationFunctionType.Exp,
                        bias=0.0, scale=0.125,
                        accum_out=dparts[:, h:h + 1],
                    )
                d = small.tile([128, 1], F32, tag="d")
                nc.vector.tensor_reduce(d, dparts, axis=mybir.AxisListType.X,
                                        op=mybir.AluOpType.add)
                rdf = small.tile([128, 1], F32, tag="rdf")
                nc.vector.reciprocal(rdf, d)
                rdb = small.tile([128, 1], BF16, tag="rdb")
                nc.vector.tensor_copy(rdb, rdf)
                # start=True zeroes the whole 2KB PSUM bank (zero-region
                # granularity), so only the very first matmul into s_psum may
                # set it; later columns of jt==0 land on pending-zero bytes and
                # overwrite, subsequent j-tiles accumulate.
                for q in range(32):
                    nc.tensor.matmul(s_psum[:, q:q + 1],
                                     P_t[:, q * 128:(q + 1) * 128], rdb,
                                     start=(jt == 0 and q == 0),
                                     stop=(jt == 15 and q == 31),
                                     skip_group_check=True)

            # ---- s -> SBUF bf16, then r12 = prsT^T @ s ----
            sT = small.tile([128, 32], BF16, tag="sT")
            nc.scalar.copy(sT, s_psum)
            r12 = acc.tile([128, 1], F32, tag="r12")
            for q in range(32):
                nc.tensor.matmul(r12[:C, :], prsT[:, q * C:(q + 1) * C], sT[:, q:q + 1],
                                 start=(q == 0), stop=(q == 31),
                                 skip_group_check=True)

            # ---- combine: out = r12 + rsum_rgb + rsum_dep ----
            tmp = small.tile([C, 1], F32, tag="tmp")
            nc.vector.tensor_reduce(tmp, rsumd, axis=mybir.AxisListType.X,
                                    op=mybir.AluOpType.add)
            nc.vector.tensor_add(tmp, tmp, rsumr)
            outsb = small.tile([C, 1], F32, tag="outsb")
            nc.vector.tensor_add(outsb, tmp, r12[:C, :])
            nc.sync.dma_start(out=out_d.ap(), in_=outsb)

    nc.compile()
    nc.finalize()
    return nc


_NC_CACHE = []


def _get_nc():
    if not _NC_CACHE:
        _NC_CACHE.append(build_core_program())
    return _NC_CACHE[0]


def make_in_maps(rgb, dep, conv_w, bn1_g, bn1_b, bn1_m, bn1_v, rgb_w, dep_w):
    B = rgb.shape[0]
    s1 = (bn1_g / np.sqrt(bn1_v + EPS)).astype(np.float32).reshape(C, 1)
    b1 = (bn1_b - bn1_m * (bn1_g / np.sqrt(bn1_v + EPS))).astype(np.float32).reshape(C, 1)
    cwT = np.ascontiguousarray(conv_w.T).astype(NPBF16)          # (256, 64)
    rgbwT = np.ascontiguousarray(rgb_w.T).astype(NPBF16)         # (64, 64)
    depwT = np.ascontiguousarray(dep_w.T).astype(NPBF16)
    in_maps = []
    for k in range(8):
        b, h = k // 2, k % 2
        off = h * NH
        r = rgb[b].reshape(C, N)
        d = dep[b].reshape(C1, N)
        r_p = np.concatenate([r[:, off:], r[:, :off]], axis=1).astype(NPBF16)
        d_p = np.concatenate([d[:, off:], d[:, :off]], axis=1).astype(NPBF16)
        in_maps.append({
            "rgb": np.ascontiguousarray(r_p),
            "depA": np.ascontiguousarray(d_p[:128]),
            "depB": np.ascontiguousarray(d_p[128:]),
            "cwT0": np.ascontiguousarray(cwT[:128]),
            "cwT1": np.ascontiguousarray(cwT[128:]),
            "rgbwT": rgbwT,
            "depwT": depwT,
            "bn1": np.ascontiguousarray(np.concatenate([s1, b1], axis=1)),
        })
    return in_maps


def kernel(rgb, dep, conv_w, bn1_g, bn1_b, bn1_m, bn1_v, rgb_w, dep_w,
           mlp1_w, bn2_g, bn2_b, bn2_m, bn2_v, mlp2_w, bn3_g, bn3_b, bn3_m, bn3_v,
           _trace=False):
    rgb = np.asarray(rgb, dtype=np.float32)
    dep = np.asarray(dep, dtype=np.float32)
    B = rgb.shape[0]
    nc = _get_nc()
    in_maps = make_in_maps(rgb, dep, conv_w, bn1_g, bn1_b, bn1_m, bn1_v, rgb_w, dep_w)
    res = run_bass_kernel_spmd(nc, in_maps, core_ids=list(range(8)), trace=_trace)
    vecs = [res.results[k]["out_vec"].reshape(C) for k in range(8)]
    gap = np.stack([(vecs[2 * b] + vecs[2 * b + 1]) / N for b in range(B)])  # (B, 64)

    # tiny gate MLP on host (part of gather)
    s2 = bn2_g / np.sqrt(bn2_v + EPS)
    bb2 = bn2_b - bn2_m * s2
    s3 = bn3_g / np.sqrt(bn3_v + EPS)
    bb3 = bn3_b - bn3_m * s3
    h = np.maximum(gap @ mlp1_w.T * s2[None, :] + bb2[None, :], 0.0)
    z = h @ mlp2_w.T * s3[None, :] + bb3[None, :]
    gate = 1.0 / (1.0 + np.exp(-z))
    out = gate.reshape(B, C, 1, 1).astype(np.float32)
    if _trace:
        kernel.last_results = res
    return out
